# revision 30
# baseline (speedup 1.0000x reference)
"""Multi-head attention (B=4, S=2048, D=1024, H=16, causal) on 8 TRN2 cores.

Sharding: core c -> (batch b = c//2, head-group g = c%2 of 8 heads).
Each core computes projections for its 8 heads (column-split Wq/Wk/Wv),
flash-style causal attention, and a partial output projection (row-split Wo).
Host unshard sums the two partials per batch and adds bo.

v2 layout/schedule notes:
- All matmul operands bf16 (PSUM accumulation stays f32); inputs cast to
  bf16 host-side so DMA volume halves and the exp pipeline starts early.
- Scores for a group of 2 key-blocks x both heads of a pair land in one
  4-bank PSUM tile [128, 2048]; exp runs as 1-2 big activations per group
  (3D APs batch same-diagonal-offset blocks) to amortize ACT overhead.
- Emission interleaves K/Q/V projections with attention units so the Tile
  scheduler always has PE filler during exp waits (keeps HAM warm).
- vw carries a ones column per head so PV (M=65) emits softmax denominators
  for free; denominators DMA straight from PSUM; per-pair normalization
  uses reciprocal_approx_fast + a K=2 broadcast matmul.
- Output projection accumulates in PSUM and DMAs directly to HBM.
"""

from contextlib import ExitStack

import numpy as np
import ml_dtypes

import concourse.bass as bass
import concourse.tile as tile
from concourse import bacc, mybir
from concourse.bass_utils import run_bass_kernel_spmd

F32 = mybir.dt.float32
F32R = mybir.dt.float32r
BF16 = mybir.dt.bfloat16
EXP = mybir.ActivationFunctionType.Exp
COPY = mybir.ActivationFunctionType.Copy

B, S, D, H = 4, 2048, 1024, 16
HD = D // H          # 64
DL = D // 2          # 512 local douts per core
NT = DL // 128       # 4 dout tiles (= head pairs)
NR = S // 128        # 16 row tiles
NQ = S // 512        # 4 query chunks
NDIN = D // 128      # 8 din tiles


def build_nc():
    nc = bacc.Bacc("TRN2", target_bir_lowering=False, debug=False, num_devices=8)

    qT = nc.dram_tensor("qT", [D, S], BF16, kind="ExternalInput").ap()
    kT = nc.dram_tensor("kT", [D, S], BF16, kind="ExternalInput").ap()
    vT = nc.dram_tensor("vT", [D, S], BF16, kind="ExternalInput").ap()
    Wq_s = nc.dram_tensor("Wq_s", [D, DL], BF16, kind="ExternalInput").ap()
    Wk_s = nc.dram_tensor("Wk_s", [D, DL], BF16, kind="ExternalInput").ap()
    Wv_s = nc.dram_tensor("Wv_s", [D, DL], BF16, kind="ExternalInput").ap()
    Wo_s = nc.dram_tensor("Wo_s", [DL, D], BF16, kind="ExternalInput").ap()
    bq_s = nc.dram_tensor("bq_s", [DL, 1], F32, kind="ExternalInput").ap()
    bk_s = nc.dram_tensor("bk_s", [DL, 1], F32, kind="ExternalInput").ap()
    bv_bc = nc.dram_tensor("bv_bc", [128, DL], F32, kind="ExternalInput").ap()
    E2_in = nc.dram_tensor("E2_in", [2, 128], F32R, kind="ExternalInput").ap()
    out_p = nc.dram_tensor("out_partial", [S, D], F32, kind="ExternalOutput").ap()

    with tile.TileContext(nc) as tc, ExitStack() as ctx:
        # ---------------- persistent SBUF ----------------
        keep = ctx.enter_context(tc.tile_pool(name="keep", bufs=1))
        qwT = [keep.tile([128, S], BF16, tag=f"qwT{t}", name=f"qwT{t}") for t in range(NT)]
        kwT = [keep.tile([128, S], BF16, tag=f"kwT{t}", name=f"kwT{t}") for t in range(NT)]
        vw = [keep.tile([128, 8 * 65], BF16, tag=f"vw{r}", name=f"vw{r}") for r in range(NR)]
        atn = [keep.tile([128, S], BF16, tag=f"atn{t}", name=f"atn{t}") for t in range(NT)]

        bias_q = keep.tile([128, NT], F32, tag="bias_q")  # col t = bq tile t
        bias_k = keep.tile([128, NT], F32, tag="bias_k")
        bv_sb = keep.tile([128, DL], F32, tag="bv_sb")
        E_sb = keep.tile([2, 128], F32R, tag="E_sb")

        # weight + slab pools (dedicated tags per tensor so DMAs overlap);
        # these close before the output projection to free PSUM banks.
        slp = ctx.enter_context(tc.tile_pool(name="slp", bufs=1))
        phase1 = ExitStack()
        wp = phase1.enter_context(tc.tile_pool(name="wp", bufs=1))
        pps = phase1.enter_context(tc.tile_pool(name="pps", bufs=2, space="PSUM"))

        for t in range(NT):
            nc.sync.dma_start(bias_q[:, t:t + 1], bq_s[128 * t:128 * (t + 1), :])
            nc.sync.dma_start(bias_k[:, t:t + 1], bk_s[128 * t:128 * (t + 1), :])
        nc.sync.dma_start(bv_sb[:], bv_bc)
        nc.sync.dma_start(E_sb[:], E2_in)
        bv3 = bv_sb[:].rearrange("p (a b) -> p a b", b=1)
        for r in range(NR):
            ones_ap = vw[r][:].rearrange("p (h e) -> p h e", e=65)[:, :, 64:65]
            nc.scalar.activation(ones_ap, bv3[:, 0:8, :], COPY, bias=1.0, scale=0.0)

        def load_w(W, pref):
            w_sb = []
            for dn in range(NDIN):
                w = wp.tile([128, DL], BF16, tag=f"{pref}{dn}")
                nc.sync.dma_start(w[:], W[128 * dn:128 * (dn + 1), :])
                w_sb.append(w)
            return w_sb

        def load_slab(xT, pref):
            sl = []
            for dn in range(NDIN):
                s_ = slp.tile([128, S], BF16, tag=f"{pref}{dn}")
                nc.sync.dma_start(s_[:], xT[128 * dn:128 * (dn + 1), :])
                sl.append(s_)
            return sl

        def proj_T_tile(w_sb, sl, bias_t, dst, t):
            # dst[t] [128, S] = tile t of (x @ W).T + bias (douts on partitions)
            for rc in range(NQ):
                ps = pps.tile([128, 512], F32, tag="pp")
                for dn in range(NDIN):
                    nc.tensor.matmul(
                        ps[:],
                        w_sb[dn][:, 128 * t:128 * (t + 1)],
                        sl[dn][:, 512 * rc:512 * (rc + 1)],
                        start=(dn == 0), stop=(dn == NDIN - 1))
                nc.vector.tensor_scalar_add(
                    dst[t][:, 512 * rc:512 * (rc + 1)],
                    ps[:], bias_t[:, t:t + 1])

        def proj_v_tile(wv_sb, slv, r):
            # vw[r] natural: [row, dout] with per-head ones column
            ps = pps.tile([128, 512], F32, tag="pp")
            for dn in range(NDIN):
                nc.tensor.matmul(
                    ps[:],
                    slv[dn][:, 128 * r:128 * (r + 1)],
                    wv_sb[dn][:],
                    start=(dn == 0), stop=(dn == NDIN - 1))
            dst3 = vw[r][:].rearrange("p (h e) -> p h e", e=65)[:, :, 0:64]
            nc.vector.tensor_add(
                dst3, ps[:].rearrange("p (h e) -> p h e", e=64),
                bv_sb[:].rearrange("p (h e) -> p h e", e=64))

        # attention-phase PSUM/SBUF pools: sc double-buffered so exp(j+1)
        # overlaps PV(j)/scores(j+1) and ACT stays saturated
        scp = phase1.enter_context(tc.tile_pool(name="scp", bufs=2, space="PSUM"))
        atp = phase1.enter_context(tc.tile_pool(name="atp", bufs=1, space="PSUM"))
        prp = phase1.enter_context(tc.tile_pool(name="prp", bufs=3))
        stp = phase1.enter_context(tc.tile_pool(name="stp", bufs=2))
        bcp = phase1.enter_context(tc.tile_pool(name="bcp", bufs=1))

        def attn_unit(p, qc):
            """Attention for head pair p, query chunk qc (512 queries)."""
            jmax = 4 * qc + 3
            qf = slice(512 * qc, 512 * (qc + 1))
            atA = atp.tile([65, 512], F32, tag="atA")
            atB = atp.tile([65, 512], F32, tag="atB")
            for j in range(jmax + 1):
                off = max(0, 128 * j - 512 * qc)
                qs = slice(512 * qc + off, 512 * (qc + 1))
                sc = scp.tile([128, 1024], F32, tag="sc")
                pr = prp.tile([128, 1024], BF16, tag="pr")
                # A head at PE rows 0-63, B head at rows 64-127 (concurrent)
                nc.tensor.matmul(
                    sc[:, off:512],
                    kwT[p][0:64, 128 * j:128 * (j + 1)],
                    qwT[p][0:64, qs],
                    start=True, stop=True, tile_position=(0, 0))
                nc.tensor.matmul(
                    sc[:, 512 + off:1024],
                    kwT[p][64:128, 128 * j:128 * (j + 1)],
                    qwT[p][64:128, qs],
                    start=True, stop=True, tile_position=(64, 0))
                # one exp instruction per block: strided over both heads
                if off == 0:
                    nc.scalar.activation(pr[:], sc[:], EXP, scale=1.0 / 8.0)
                else:
                    sc2 = sc[:].rearrange("p (b c) -> p b c", c=512)
                    pr2 = pr[:].rearrange("p (b c) -> p b c", c=512)
                    nc.scalar.activation(
                        pr2[:, :, off:512], sc2[:, :, off:512],
                        EXP, scale=1.0 / 8.0)
                # causal mask on diagonal block (zero probs above diagonal)
                if 128 * j >= 512 * qc:
                    for side in range(2):
                        c0 = 512 * side + off
                        nc.gpsimd.affine_select(
                            out=pr[:, c0:c0 + 128],
                            in_=pr[:, c0:c0 + 128],
                            channel_multiplier=-1,
                            pattern=[[1, 128]], base=0,
                            compare_op=mybir.AluOpType.is_ge,
                            fill=0.0)
                # PV accumulation (M=65: ones column gives denominators)
                nc.tensor.matmul(
                    atA[0:65, off:512],
                    vw[j][:, 65 * 2 * p:65 * 2 * p + 65],
                    pr[:, off:512],
                    start=(j == 0), stop=(j == jmax))
                nc.tensor.matmul(
                    atB[0:65, off:512],
                    vw[j][:, 65 * (2 * p + 1):65 * (2 * p + 1) + 65],
                    pr[:, 512 + off:1024],
                    start=(j == 0), stop=(j == jmax))
            # drain PSUM fast (unnormalized copy releases atA/atB for the
            # next unit), then normalize atn in place off the critical path:
            # broadcast denominators via K=1 matmuls, 128-lane approx recip.
            ones64 = E_sb[0:1, 0:64]
            for side, (at_, rows) in enumerate(
                    [(atA, slice(0, 64)), (atB, slice(64, 128))]):
                stg = stp.tile([1, 512], F32R, tag=f"stg{side}")
                nc.vector.tensor_copy(stg[:], at_[64:65, :])
                nc.vector.tensor_copy(atn[p][rows, qf], at_[0:64, :])
                bcd = pps.tile([64, 512], F32, tag="pp")
                nc.tensor.matmul(bcd[:], ones64, stg[:],
                                 start=True, stop=True)
                bcr = bcp.tile([64, 512], F32, tag=f"bcr{side}")
                nc.vector.reciprocal_approx_fast(bcr[:], bcd[:])
                nc.vector.tensor_mul(atn[p][rows, qf], atn[p][rows, qf],
                                     bcr[:])

        # ---------------- emission schedule ----------------
        # Emission order = scheduler priority. Attention units are emitted
        # BEFORE the projections that serve as PE filler: the greedy
        # scheduler runs blocked-attention prerequisites on demand and uses
        # lower-priority projection matmuls to fill exp-wait gaps.
        wk_sb = load_w(Wk_s, "wk")
        slk = load_slab(kT, "slk")
        wq_sb = load_w(Wq_s, "wq")
        slq = load_slab(qT, "slq")
        wv_sb = load_w(Wv_s, "wv")
        slv = load_slab(vT, "slv")

        proj_T_tile(wk_sb, slk, bias_k, kwT, 0)
        proj_T_tile(wq_sb, slq, bias_q, qwT, 0)
        for r in range(4):
            proj_v_tile(wv_sb, slv, r)
        for qc in range(NQ):
            attn_unit(0, qc)
        for r in range(4, NR):
            proj_v_tile(wv_sb, slv, r)
        for p in range(1, NT):
            proj_T_tile(wk_sb, slk, bias_k, kwT, p)
            proj_T_tile(wq_sb, slq, bias_q, qwT, p)
            for qc in range(NQ):
                attn_unit(p, qc)
        # wo reuses the Q slab slots (all Q proj reads done by now)
        wo_sb = []
        for t in range(NT):
            w = slp.tile([128, D], BF16, tag=f"slq{t}")
            nc.sync.dma_start(w[:], Wo_s[128 * t:128 * (t + 1), :])
            wo_sb.append(w)

        phase1.close()

        # ---------------- output projection ----------------
        with tc.tile_pool(name="ops", bufs=2, space="PSUM") as opp, \
             tc.tile_pool(name="osb", bufs=2) as osp:
            for rt in range(NR):
                po = opp.tile([128, D], F32, tag="po")
                for nch in range(2):
                    for t in range(NT):
                        nc.tensor.matmul(
                            po[:, 512 * nch:512 * (nch + 1)],
                            atn[t][:, 128 * rt:128 * (rt + 1)],
                            wo_sb[t][:, 512 * nch:512 * (nch + 1)],
                            start=(t == 0), stop=(t == NT - 1))
                ob = osp.tile([128, D], F32, tag="ob")
                # alternate ACT/DVE for the PSUM->SBUF drain (both idle here)
                if rt % 2 == 0:
                    nc.vector.tensor_copy(ob[:], po[:])
                else:
                    nc.scalar.copy(ob[:], po[:])
                nc.sync.dma_start(out_p[128 * rt:128 * (rt + 1), :], ob[:])

    nc.compile()
    return nc


_NC_CACHE = {}


def get_nc():
    if "nc" not in _NC_CACHE:
        _NC_CACHE["nc"] = build_nc()
    return _NC_CACHE["nc"]


def make_in_maps(q, k, v, Wq, bq, Wk, bk, Wv, bv, Wo):
    """Host-side shard prep. Returns list of 8 per-core input dicts."""
    f = np.float32
    bf = ml_dtypes.bfloat16
    q = np.asarray(q, f)
    k = np.asarray(k, f)
    v = np.asarray(v, f)
    Wq, bq = np.asarray(Wq, f), np.asarray(bq, f)
    Wk, bk = np.asarray(Wk, f), np.asarray(bk, f)
    Wv, bv = np.asarray(Wv, f), np.asarray(bv, f)
    Wo = np.asarray(Wo, f)
    E2 = np.zeros((2, 128), f)
    E2[0, 0:64] = 1.0
    E2[1, 64:128] = 1.0
    qTb = [np.ascontiguousarray(q[b].T).astype(bf) for b in range(B)]
    kTb = [np.ascontiguousarray(k[b].T).astype(bf) for b in range(B)]
    vTb = [np.ascontiguousarray(v[b].T).astype(bf) for b in range(B)]
    in_maps = []
    for c in range(8):
        b, g = c // 2, c % 2
        cs = slice(DL * g, DL * (g + 1))
        in_maps.append(dict(
            qT=qTb[b],
            kT=kTb[b],
            vT=vTb[b],
            Wq_s=np.ascontiguousarray(Wq[:, cs]).astype(bf),
            Wk_s=np.ascontiguousarray(Wk[:, cs]).astype(bf),
            Wv_s=np.ascontiguousarray(Wv[:, cs]).astype(bf),
            Wo_s=np.ascontiguousarray(Wo[cs, :]).astype(bf),
            bq_s=np.ascontiguousarray(bq[cs]).reshape(DL, 1),
            bk_s=np.ascontiguousarray(bk[cs]).reshape(DL, 1),
            bv_bc=np.tile(bv[cs][None, :], (128, 1)),
            E2_in=E2,
        ))
    return in_maps


def unshard(results, bo):
    bo = np.asarray(bo, np.float32)
    out = np.empty((B, S, D), np.float32)
    for b in range(B):
        out[b] = (results[2 * b]["out_partial"]
                  + results[2 * b + 1]["out_partial"] + bo)
    return out


def kernel(q, k, v, mask, Wq, bq, Wk, bk, Wv, bv, Wo, bo, **_unused):
    nc = get_nc()
    in_maps = make_in_maps(q, k, v, Wq, bq, Wk, bk, Wv, bv, Wo)
    res = run_bass_kernel_spmd(nc, in_maps, core_ids=list(range(8))).results
    return unshard(res, bo)


# revision 35
# speedup vs baseline: 1.0195x; 1.0195x over previous
"""Multi-head attention (B=4, S=2048, D=1024, H=16, causal) on 8 TRN2 cores.

Sharding: core c -> (batch b = c//2, head-group g = c%2 of 8 heads).
Each core computes projections for its 8 heads (column-split Wq/Wk/Wv),
flash-style causal attention, and a partial output projection (row-split Wo).
Host unshard sums the two partials per batch and adds bo.

v2 layout/schedule notes:
- All matmul operands bf16 (PSUM accumulation stays f32); inputs cast to
  bf16 host-side so DMA volume halves and the exp pipeline starts early.
- Scores for a group of 2 key-blocks x both heads of a pair land in one
  4-bank PSUM tile [128, 2048]; exp runs as 1-2 big activations per group
  (3D APs batch same-diagonal-offset blocks) to amortize ACT overhead.
- Emission interleaves K/Q/V projections with attention units so the Tile
  scheduler always has PE filler during exp waits (keeps HAM warm).
- vw carries a ones column per head so PV (M=65) emits softmax denominators
  for free; denominators DMA straight from PSUM; per-pair normalization
  uses reciprocal_approx_fast + a K=2 broadcast matmul.
- Output projection accumulates in PSUM and DMAs directly to HBM.
"""

from contextlib import ExitStack

import numpy as np
import ml_dtypes

import concourse.bass as bass
import concourse.tile as tile
from concourse import bacc, mybir
from concourse.bass_utils import run_bass_kernel_spmd

F32 = mybir.dt.float32
F32R = mybir.dt.float32r
BF16 = mybir.dt.bfloat16
EXP = mybir.ActivationFunctionType.Exp
COPY = mybir.ActivationFunctionType.Copy

B, S, D, H = 4, 2048, 1024, 16
HD = D // H          # 64
DL = D // 2          # 512 local douts per core
NT = DL // 128       # 4 dout tiles (= head pairs)
NR = S // 128        # 16 row tiles
NQ = S // 512        # 4 query chunks
NDIN = D // 128      # 8 din tiles


def build_nc():
    nc = bacc.Bacc("TRN2", target_bir_lowering=False, debug=False, num_devices=8)

    qT = nc.dram_tensor("qT", [D, S], BF16, kind="ExternalInput").ap()
    kT = nc.dram_tensor("kT", [D, S], BF16, kind="ExternalInput").ap()
    vT = nc.dram_tensor("vT", [D, S], BF16, kind="ExternalInput").ap()
    Wq_s = nc.dram_tensor("Wq_s", [D, DL], BF16, kind="ExternalInput").ap()
    Wk_s = nc.dram_tensor("Wk_s", [D, DL], BF16, kind="ExternalInput").ap()
    Wv_s = nc.dram_tensor("Wv_s", [D, DL], BF16, kind="ExternalInput").ap()
    Wo_s = nc.dram_tensor("Wo_s", [DL, D], BF16, kind="ExternalInput").ap()
    bq_s = nc.dram_tensor("bq_s", [DL, 1], F32, kind="ExternalInput").ap()
    bk_s = nc.dram_tensor("bk_s", [DL, 1], F32, kind="ExternalInput").ap()
    bv_bc = nc.dram_tensor("bv_bc", [128, DL], F32, kind="ExternalInput").ap()
    E2_in = nc.dram_tensor("E2_in", [2, 128], F32R, kind="ExternalInput").ap()
    out_p = nc.dram_tensor("out_partial", [S, D], F32, kind="ExternalOutput").ap()

    with tile.TileContext(nc) as tc, ExitStack() as ctx:
        # ---------------- persistent SBUF ----------------
        keep = ctx.enter_context(tc.tile_pool(name="keep", bufs=1))
        qwT = [keep.tile([128, S], BF16, tag=f"qwT{t}", name=f"qwT{t}") for t in range(NT)]
        kwT = [keep.tile([128, S], BF16, tag=f"kwT{t}", name=f"kwT{t}") for t in range(NT)]
        vw = [keep.tile([128, 8 * 65], BF16, tag=f"vw{r}", name=f"vw{r}") for r in range(NR)]
        atn = [keep.tile([128, S], BF16, tag=f"atn{t}", name=f"atn{t}") for t in range(NT)]

        bias_q = keep.tile([128, NT], F32, tag="bias_q")  # col t = bq tile t
        bias_k = keep.tile([128, NT], F32, tag="bias_k")
        bv_sb = keep.tile([128, DL], F32, tag="bv_sb")
        E_A = keep.tile([1, 128], F32R, tag="E_A")  # ones at cols 0-63
        E_B = keep.tile([1, 128], F32R, tag="E_B")  # ones at cols 64-127

        # weight + slab pools (dedicated tags per tensor so DMAs overlap);
        # these close before the output projection to free PSUM banks.
        slp = ctx.enter_context(tc.tile_pool(name="slp", bufs=1))
        phase1 = ExitStack()
        wp = phase1.enter_context(tc.tile_pool(name="wp", bufs=1))
        pps = phase1.enter_context(tc.tile_pool(name="pps", bufs=2, space="PSUM"))

        for t in range(NT):
            nc.sync.dma_start(bias_q[:, t:t + 1], bq_s[128 * t:128 * (t + 1), :])
            nc.sync.dma_start(bias_k[:, t:t + 1], bk_s[128 * t:128 * (t + 1), :])
        nc.sync.dma_start(bv_sb[:], bv_bc)
        nc.sync.dma_start(E_A[:], E2_in[0:1, :])
        nc.sync.dma_start(E_B[:], E2_in[1:2, :])
        bv3 = bv_sb[:].rearrange("p (a b) -> p a b", b=1)
        for r in range(NR):
            ones_ap = vw[r][:].rearrange("p (h e) -> p h e", e=65)[:, :, 64:65]
            nc.scalar.activation(ones_ap, bv3[:, 0:8, :], COPY, bias=1.0, scale=0.0)

        def load_w(W, pref):
            w_sb = []
            for dn in range(NDIN):
                w = wp.tile([128, DL], BF16, tag=f"{pref}{dn}")
                nc.sync.dma_start(w[:], W[128 * dn:128 * (dn + 1), :])
                w_sb.append(w)
            return w_sb

        def load_slab(xT, pref):
            sl = []
            for dn in range(NDIN):
                s_ = slp.tile([128, S], BF16, tag=f"{pref}{dn}")
                nc.sync.dma_start(s_[:], xT[128 * dn:128 * (dn + 1), :])
                sl.append(s_)
            return sl

        def proj_T_tile(w_sb, sl, bias_t, dst, t):
            # dst[t] [128, S] = tile t of (x @ W).T + bias (douts on partitions)
            for rc in range(NQ):
                ps = pps.tile([128, 512], F32, tag="pp")
                for dn in range(NDIN):
                    nc.tensor.matmul(
                        ps[:],
                        w_sb[dn][:, 128 * t:128 * (t + 1)],
                        sl[dn][:, 512 * rc:512 * (rc + 1)],
                        start=(dn == 0), stop=(dn == NDIN - 1))
                nc.vector.tensor_scalar_add(
                    dst[t][:, 512 * rc:512 * (rc + 1)],
                    ps[:], bias_t[:, t:t + 1])

        def proj_v_tile(wv_sb, slv, r):
            # vw[r] natural: [row, dout] with per-head ones column
            ps = pps.tile([128, 512], F32, tag="pp")
            for dn in range(NDIN):
                nc.tensor.matmul(
                    ps[:],
                    slv[dn][:, 128 * r:128 * (r + 1)],
                    wv_sb[dn][:],
                    start=(dn == 0), stop=(dn == NDIN - 1))
            dst3 = vw[r][:].rearrange("p (h e) -> p h e", e=65)[:, :, 0:64]
            nc.vector.tensor_add(
                dst3, ps[:].rearrange("p (h e) -> p h e", e=64),
                bv_sb[:].rearrange("p (h e) -> p h e", e=64))

        # attention-phase PSUM/SBUF pools: sc double-buffered so exp(j+1)
        # overlaps PV(j)/scores(j+1) and ACT stays saturated
        scp = phase1.enter_context(tc.tile_pool(name="scp", bufs=2, space="PSUM"))
        atp = phase1.enter_context(tc.tile_pool(name="atp", bufs=1, space="PSUM"))
        prp = phase1.enter_context(tc.tile_pool(name="prp", bufs=3))
        stp = phase1.enter_context(tc.tile_pool(name="stp", bufs=2))
        bcp = phase1.enter_context(tc.tile_pool(name="bcp", bufs=1))

        def attn_unit(p, qc):
            """Attention for head pair p, query chunk qc (512 queries)."""
            jmax = 4 * qc + 3
            qf = slice(512 * qc, 512 * (qc + 1))
            atA = atp.tile([65, 512], F32, tag="atA")
            atB = atp.tile([65, 512], F32, tag="atB")
            for j in range(jmax + 1):
                off = max(0, 128 * j - 512 * qc)
                qs = slice(512 * qc + off, 512 * (qc + 1))
                sc = scp.tile([128, 1024], F32, tag="sc")
                pr = prp.tile([128, 1024], BF16, tag="pr")
                # A head at PE rows 0-63, B head at rows 64-127 (concurrent)
                nc.tensor.matmul(
                    sc[:, off:512],
                    kwT[p][0:64, 128 * j:128 * (j + 1)],
                    qwT[p][0:64, qs],
                    start=True, stop=True, tile_position=(0, 0))
                nc.tensor.matmul(
                    sc[:, 512 + off:1024],
                    kwT[p][64:128, 128 * j:128 * (j + 1)],
                    qwT[p][64:128, qs],
                    start=True, stop=True, tile_position=(64, 0))
                # one exp instruction per block: strided over both heads
                if off == 0:
                    nc.scalar.activation(pr[:], sc[:], EXP, scale=1.0 / 8.0)
                else:
                    sc2 = sc[:].rearrange("p (b c) -> p b c", c=512)
                    pr2 = pr[:].rearrange("p (b c) -> p b c", c=512)
                    nc.scalar.activation(
                        pr2[:, :, off:512], sc2[:, :, off:512],
                        EXP, scale=1.0 / 8.0)
                # causal mask on diagonal block (zero probs above diagonal)
                if 128 * j >= 512 * qc:
                    for side in range(2):
                        c0 = 512 * side + off
                        nc.gpsimd.affine_select(
                            out=pr[:, c0:c0 + 128],
                            in_=pr[:, c0:c0 + 128],
                            channel_multiplier=-1,
                            pattern=[[1, 128]], base=0,
                            compare_op=mybir.AluOpType.is_ge,
                            fill=0.0)
                # PV accumulation (M=65: ones column gives denominators)
                nc.tensor.matmul(
                    atA[0:65, off:512],
                    vw[j][:, 65 * 2 * p:65 * 2 * p + 65],
                    pr[:, off:512],
                    start=(j == 0), stop=(j == jmax))
                nc.tensor.matmul(
                    atB[0:65, off:512],
                    vw[j][:, 65 * (2 * p + 1):65 * (2 * p + 1) + 65],
                    pr[:, 512 + off:1024],
                    start=(j == 0), stop=(j == jmax))
            # drain PSUM fast (unnormalized copy releases atA/atB for the
            # next unit), then normalize atn in place off the critical path:
            # broadcast denominators via K=1 matmuls, 128-lane approx recip.
            stgA = stp.tile([1, 512], F32R, tag="stg0")
            stgB = stp.tile([1, 512], F32R, tag="stg1")
            nc.vector.tensor_copy(stgA[:], atA[64:65, :])
            nc.vector.tensor_copy(stgB[:], atB[64:65, :])
            nc.vector.tensor_copy(atn[p][0:64, qf], atA[0:64, :])
            nc.vector.tensor_copy(atn[p][64:128, qf], atB[0:64, :])
            # accumulate [denomA rows 0-63; denomB rows 64-127] via two K=1
            # matmuls (ones-selector vectors), then one aligned reciprocal
            bcd = pps.tile([128, 512], F32, tag="pp")
            nc.tensor.matmul(bcd[:], E_A[:], stgA[:], start=True, stop=False)
            nc.tensor.matmul(bcd[:], E_B[:], stgB[:], start=False, stop=True)
            bcr = bcp.tile([128, 512], F32, tag="bcr")
            nc.vector.reciprocal_approx_fast(bcr[:], bcd[:])
            nc.vector.tensor_mul(atn[p][0:64, qf], atn[p][0:64, qf],
                                 bcr[0:64, :])
            nc.vector.tensor_mul(atn[p][64:128, qf], atn[p][64:128, qf],
                                 bcr[64:128, :])

        # ---------------- emission schedule ----------------
        # Emission must be producer-first (Tile tracks deps in trace order),
        # so projections are interleaved just-in-time before the attention
        # units that need them. Attention (incl. its normalize chain) is
        # wrapped in high_priority so the scheduler treats the projections
        # emitted earlier as low-priority PE filler during exp waits.
        def hi_attn(p, qc):
            with tc.high_priority():
                attn_unit(p, qc)

        wk_sb = load_w(Wk_s, "wk")
        slk = load_slab(kT, "slk")
        wq_sb = load_w(Wq_s, "wq")
        slq = load_slab(qT, "slq")
        wv_sb = load_w(Wv_s, "wv")
        slv = load_slab(vT, "slv")

        proj_T_tile(wk_sb, slk, bias_k, kwT, 0)
        proj_T_tile(wq_sb, slq, bias_q, qwT, 0)
        for r in range(4):
            proj_v_tile(wv_sb, slv, r)
        hi_attn(0, 0)
        for r in range(4, 8):
            proj_v_tile(wv_sb, slv, r)
        hi_attn(0, 1)
        for r in range(8, 12):
            proj_v_tile(wv_sb, slv, r)
        hi_attn(0, 2)
        for r in range(12, 16):
            proj_v_tile(wv_sb, slv, r)
        hi_attn(0, 3)
        for p in range(1, NT):
            proj_T_tile(wk_sb, slk, bias_k, kwT, p)
            proj_T_tile(wq_sb, slq, bias_q, qwT, p)
            for qc in range(NQ):
                hi_attn(p, qc)
        # wo reuses the Q slab slots (all Q proj reads done by now)
        wo_sb = []
        for t in range(NT):
            w = slp.tile([128, D], BF16, tag=f"slq{t}")
            nc.sync.dma_start(w[:], Wo_s[128 * t:128 * (t + 1), :])
            wo_sb.append(w)

        phase1.close()

        # ---------------- output projection ----------------
        with tc.tile_pool(name="ops", bufs=2, space="PSUM") as opp, \
             tc.tile_pool(name="osb", bufs=2) as osp:
            for rt in range(NR):
                po = opp.tile([128, D], F32, tag="po")
                for nch in range(2):
                    for t in range(NT):
                        nc.tensor.matmul(
                            po[:, 512 * nch:512 * (nch + 1)],
                            atn[t][:, 128 * rt:128 * (rt + 1)],
                            wo_sb[t][:, 512 * nch:512 * (nch + 1)],
                            start=(t == 0), stop=(t == NT - 1))
                ob = osp.tile([128, D], F32, tag="ob")
                # alternate ACT/DVE for the PSUM->SBUF drain (both idle here)
                if rt % 2 == 0:
                    nc.vector.tensor_copy(ob[:], po[:])
                else:
                    nc.scalar.copy(ob[:], po[:])
                nc.sync.dma_start(out_p[128 * rt:128 * (rt + 1), :], ob[:])

    nc.compile()
    return nc


_NC_CACHE = {}


def get_nc():
    if "nc" not in _NC_CACHE:
        _NC_CACHE["nc"] = build_nc()
    return _NC_CACHE["nc"]


def make_in_maps(q, k, v, Wq, bq, Wk, bk, Wv, bv, Wo):
    """Host-side shard prep. Returns list of 8 per-core input dicts."""
    f = np.float32
    bf = ml_dtypes.bfloat16
    q = np.asarray(q, f)
    k = np.asarray(k, f)
    v = np.asarray(v, f)
    Wq, bq = np.asarray(Wq, f), np.asarray(bq, f)
    Wk, bk = np.asarray(Wk, f), np.asarray(bk, f)
    Wv, bv = np.asarray(Wv, f), np.asarray(bv, f)
    Wo = np.asarray(Wo, f)
    E2 = np.zeros((2, 128), f)
    E2[0, 0:64] = 1.0
    E2[1, 64:128] = 1.0
    qTb = [np.ascontiguousarray(q[b].T).astype(bf) for b in range(B)]
    kTb = [np.ascontiguousarray(k[b].T).astype(bf) for b in range(B)]
    vTb = [np.ascontiguousarray(v[b].T).astype(bf) for b in range(B)]
    in_maps = []
    for c in range(8):
        b, g = c // 2, c % 2
        cs = slice(DL * g, DL * (g + 1))
        in_maps.append(dict(
            qT=qTb[b],
            kT=kTb[b],
            vT=vTb[b],
            Wq_s=np.ascontiguousarray(Wq[:, cs]).astype(bf),
            Wk_s=np.ascontiguousarray(Wk[:, cs]).astype(bf),
            Wv_s=np.ascontiguousarray(Wv[:, cs]).astype(bf),
            Wo_s=np.ascontiguousarray(Wo[cs, :]).astype(bf),
            bq_s=np.ascontiguousarray(bq[cs]).reshape(DL, 1),
            bk_s=np.ascontiguousarray(bk[cs]).reshape(DL, 1),
            bv_bc=np.tile(bv[cs][None, :], (128, 1)),
            E2_in=E2,
        ))
    return in_maps


def unshard(results, bo):
    bo = np.asarray(bo, np.float32)
    out = np.empty((B, S, D), np.float32)
    for b in range(B):
        out[b] = (results[2 * b]["out_partial"]
                  + results[2 * b + 1]["out_partial"] + bo)
    return out


def kernel(q, k, v, mask, Wq, bq, Wk, bk, Wv, bv, Wo, bo, **_unused):
    nc = get_nc()
    in_maps = make_in_maps(q, k, v, Wq, bq, Wk, bk, Wv, bv, Wo)
    res = run_bass_kernel_spmd(nc, in_maps, core_ids=list(range(8))).results
    return unshard(res, bo)
